# revision 1
# baseline (speedup 1.0000x reference)
"""CrossAttention TRN2 kernel: 8-core SPMD, shard = (batch, S1-half).

Per core: q rows [1024, 512] of one batch; full k,v [2048,512] of that batch;
all weights. Flash-style attention with S^T recompute (no P transpose):
  1. PE-transpose q,k,v -> qT,kT,vT (d on partitions), round to fp32r.
  2. Projections (fp32r): qhT_aug[65,1024]/khT_aug[65,2048] per head,
     vh[t,512] bf16 (all heads).
  3. Per head: raw-S max pass ([s,t] psum, DVE negated rowmax) -> aug row
     of qhT (-max, via HWDGE sbuf-to-sbuf DMA); S^T pass with K=65 (ones row in khT
     adds -max[s]); ACT exp(scale=1/8) -> P^T bf16; PV col-packed with
     ones-lhsT row-sum matmul (concurrent col group) accumulating
     oT[65, s] psum over 16 t-tiles.
  4. Per head: transpose oT+r -> [s, 64|r], reciprocal+scale (normalize),
     assemble out_norm [s, 512]; transpose back -> outT [hp, s];
     final proj vs Wo (bf16) -> out [1024, 512].
"""
import sys
import functools

sys.path.insert(0, "/opt/trn_rl_repo")
import numpy as np
from contextlib import ExitStack

B, S1, S2, D, H, P = 4, 2048, 2048, 512, 8, 64
SC = S1 // 2          # 1024 q rows per core
NCORES = 8
DCH = D // 128        # 4 d-chunks
QT = SC // 128        # 8 q s-tiles
TT = S2 // 128        # 16 t-tiles
TBLK = S2 // 512      # 4 t blocks of 512
SBL = SC // 512       # 2 s blocks of 512


@functools.lru_cache(maxsize=1)
def _build():
    from concourse import bacc, tile, mybir, masks

    f32 = mybir.dt.float32
    f32r = mybir.dt.float32r
    bf16 = mybir.dt.bfloat16

    nc = bacc.Bacc("TRN2", target_bir_lowering=False, debug=False)

    q_d = nc.dram_tensor("q", [SC, D], f32, kind="ExternalInput").ap()
    k_d = nc.dram_tensor("k", [S2, D], f32, kind="ExternalInput").ap()
    v_d = nc.dram_tensor("v", [S2, D], f32, kind="ExternalInput").ap()
    wq_d = nc.dram_tensor("Wq", [H, D, P], f32, kind="ExternalInput").ap()
    wk_d = nc.dram_tensor("Wk", [H, D, P], f32, kind="ExternalInput").ap()
    wv_d = nc.dram_tensor("Wv", [H, D, P], f32, kind="ExternalInput").ap()
    wo_d = nc.dram_tensor("Wo", [H * P, D], f32, kind="ExternalInput").ap()
    out_d = nc.dram_tensor("out", [SC, D], f32, kind="ExternalOutput").ap()

    with tile.TileContext(nc) as tc, ExitStack() as ctx:
        const_pool = ctx.enter_context(tc.tile_pool(name="const", bufs=1))
        ident = const_pool.tile([128, 128], f32)
        masks.make_identity(nc, ident[:])
        ident_bf = const_pool.tile([128, 128], bf16)
        masks.make_identity(nc, ident_bf[:])
        ones_bf = const_pool.tile([128, 1], bf16)
        nc.vector.memset(ones_bf[:], 1.0)

        # ---- weights: load fp32, round to fp32r on gpsimd / cast bf16 ----
        wpool = ctx.enter_context(tc.tile_pool(name="wr", bufs=1))
        wq_r = [wpool.tile([128, H * P], f32r, tag=f"wq{c}", name=f"wq{c}") for c in range(DCH)]
        wk_r = [wpool.tile([128, H * P], f32r, tag=f"wk{c}", name=f"wk{c}") for c in range(DCH)]
        wv_r = [wpool.tile([128, H * P], f32r, tag=f"wv{c}", name=f"wv{c}") for c in range(DCH)]
        wo_bf = [wpool.tile([128, D], bf16, tag=f"wo{c}", name=f"wo{c}") for c in range(DCH)]

        nat_pool = ctx.enter_context(tc.tile_pool(name="nat", bufs=8))
        act_pool = ctx.enter_context(tc.tile_pool(name="acts", bufs=1))
        qhT = [act_pool.tile([65, SC], f32r, tag=f"qhT{h}", name=f"qhT{h}") for h in range(H)]
        khT = [act_pool.tile([65, S2], f32r, tag=f"khT{h}", name=f"khT{h}") for h in range(H)]
        vh = [act_pool.tile([128, H * P], bf16, tag=f"vh{t}", name=f"vh{t}") for t in range(TT)]
        ones_row = const_pool.tile([1, S2], f32)
        nc.vector.memset(ones_row[:], 1.0)
        for h in range(H):
            nc.scalar.copy(khT[h][64:65, :], ones_row[:])

        def transpose_round(src_d, nrows, dstT):
            """src_d [nrows, D] fp32 DRAM -> dstT[c] [128, nrows] fp32r (c = d-chunk)."""
            with tc.tile_pool(name="tp_ps", bufs=2, space="PSUM") as tp_ps:
                ntile = nrows // 128
                for g in range(ntile // 4):
                    nats = []
                    for j in range(4):
                        si = 4 * g + j
                        nat = nat_pool.tile([128, D], f32, tag="nat")
                        nc.sync.dma_start(nat[:], src_d[si * 128:(si + 1) * 128, :])
                        nats.append(nat)
                    for c in range(DCH):
                        ps = tp_ps.tile([128, 512], f32)
                        for j in range(4):
                            nc.tensor.transpose(
                                ps[:, j * 128:(j + 1) * 128],
                                nats[j][:, c * 128:(c + 1) * 128],
                                ident[:],
                            )
                        nc.vector.tensor_copy(dstT[c][:, g * 512:(g + 1) * 512], ps[:])

        # ---- q path ----
        with tc.tile_pool(name="qT", bufs=1) as qT_pool, \
             tc.tile_pool(name="proj_ps", bufs=3, space="PSUM") as proj_ps:
            qT = [qT_pool.tile([128, SC], f32r, tag=f"qT{c}", name=f"qT{c}") for c in range(DCH)]
            transpose_round(q_d, SC, qT)
            with tc.tile_pool(name="wtmp", bufs=1) as wtmp_pool:
                for name_d, dst in ((wq_d, wq_r), (wk_d, wk_r), (wv_d, wv_r)):
                    for c in range(DCH):
                        wt = wtmp_pool.tile([128, H * P], f32, tag=f"wt{c}")
                        for h in range(H):
                            nc.sync.dma_start(
                                wt[:, h * P:(h + 1) * P],
                                name_d[h, c * 128:(c + 1) * 128, :],
                            )
                        nc.gpsimd.tensor_copy(dst[c][:], wt[:])
                for c in range(DCH):
                    wt = wtmp_pool.tile([128, D], f32, tag=f"wtmp_o{c}")
                    nc.sync.dma_start(wt[:], wo_d[c * 128:(c + 1) * 128, :])
                    nc.gpsimd.tensor_copy(wo_bf[c][:], wt[:])
            for hp in range(H // 2):
                for sb in range(SBL):
                    ps = proj_ps.tile([128, 512], f32)
                    for c in range(DCH):
                        nc.tensor.matmul(
                            ps[:],
                            wq_r[c][:, hp * 128:(hp + 1) * 128],
                            qT[c][:, sb * 512:(sb + 1) * 512],
                            start=(c == 0), stop=(c == DCH - 1),
                        )
                    eng = nc.scalar.copy if sb == 0 else nc.vector.tensor_copy
                    eng(qhT[2 * hp][0:64, sb * 512:(sb + 1) * 512], ps[0:64, :])
                    eng(qhT[2 * hp + 1][0:64, sb * 512:(sb + 1) * 512], ps[64:128, :])

        # ---- k path ----
        with tc.tile_pool(name="kT", bufs=1) as kT_pool, \
             tc.tile_pool(name="proj_ps2", bufs=3, space="PSUM") as proj_ps:
            kT = [kT_pool.tile([128, S2], f32r, tag=f"kT{c}", name=f"kT{c}") for c in range(DCH)]
            transpose_round(k_d, S2, kT)
            for hp in range(H // 2):
                for tb in range(TBLK):
                    ps = proj_ps.tile([128, 512], f32)
                    for c in range(DCH):
                        nc.tensor.matmul(
                            ps[:],
                            wk_r[c][:, hp * 128:(hp + 1) * 128],
                            kT[c][:, tb * 512:(tb + 1) * 512],
                            start=(c == 0), stop=(c == DCH - 1),
                        )
                    eng = nc.scalar.copy if tb % 2 == 0 else nc.vector.tensor_copy
                    eng(khT[2 * hp][0:64, tb * 512:(tb + 1) * 512], ps[0:64, :])
                    eng(khT[2 * hp + 1][0:64, tb * 512:(tb + 1) * 512], ps[64:128, :])

        # ---- v path ----
        with tc.tile_pool(name="vT", bufs=1) as vT_pool, \
             tc.tile_pool(name="proj_ps3", bufs=3, space="PSUM") as proj_ps:
            vT = [vT_pool.tile([128, S2], f32r, tag=f"vT{c}", name=f"vT{c}") for c in range(DCH)]
            transpose_round(v_d, S2, vT)
            for ti in range(TT):
                ps = proj_ps.tile([128, 512], f32)
                for c in range(DCH):
                    nc.tensor.matmul(
                        ps[:],
                        vT[c][:, ti * 128:(ti + 1) * 128],
                        wv_r[c][:],
                        start=(c == 0), stop=(c == DCH - 1),
                    )
                nc.vector.tensor_copy(vh[ti][:], ps[:])

        # ---- attention per head ----
        fin_pool = ctx.enter_context(tc.tile_pool(name="fin", bufs=1))
        out_norm = [fin_pool.tile([128, H * P], bf16, tag=f"onorm{sc}", name=f"onorm{sc}") for sc in range(QT)]
        outT = [fin_pool.tile([128, SC], bf16, tag=f"outT{c}", name=f"outT{c}") for c in range(DCH)]

        with tc.tile_pool(name="max_ps", bufs=1, space="PSUM") as max_ps, \
             tc.tile_pool(name="st_ps", bufs=2, space="PSUM") as st_ps, \
             tc.tile_pool(name="oT_ps", bufs=2, space="PSUM") as oT_ps, \
             tc.tile_pool(name="pt", bufs=3) as pt_pool, \
             tc.tile_pool(name="small", bufs=8) as small_pool, \
             tc.tile_pool(name="oT_sb", bufs=2) as oT_sb_pool:
            def maxpass_qi(h, qi):
                negm = small_pool.tile([128, 1], f32, tag="negm", name=f"negm{h}_{qi}")
                tmp = small_pool.tile([128, 1], f32, tag="tmpm", name=f"tmpm{h}_{qi}")
                negm_r = small_pool.tile([128, 1], f32r, tag="negmr", name=f"negmr{h}_{qi}")
                for half in range(2):
                    ps = max_ps.tile([128, 1024], f32, tag="mx", name=f"mx{h}_{qi}_{half}")
                    for tb in range(2):
                        nc.tensor.matmul(
                            ps[:, tb * 512:(tb + 1) * 512],
                            qhT[h][0:64, qi * 128:(qi + 1) * 128],
                            khT[h][0:64, (2 * half + tb) * 512:(2 * half + tb + 1) * 512],
                            start=True, stop=True,
                        )
                    dst = negm if half == 0 else tmp
                    nc.vector.tensor_reduce(
                        dst[:], ps[:], axis=mybir.AxisListType.X,
                        op=mybir.AluOpType.max, negate=True,
                    )
                nc.vector.tensor_scalar_min(negm[:], tmp[:], negm[:])
                nc.vector.tensor_copy(negm_r[:], negm[:])
                nc.sync.dma_start(
                    qhT[h][64:65, qi * 128:(qi + 1) * 128], negm_r[:],
                )

            for qi in range(QT):
                maxpass_qi(0, qi)

            for h in range(H):
                # --- S^T + exp + PV/rowsum, next head's max pass interleaved ---
                oTs = [oT_ps.tile([65, 512], f32, tag="oT", name=f"oT{h}_{_sb}") for _sb in range(SBL)]
                for ti in range(TT):
                    st = st_ps.tile([128, 1024], f32, tag="st", name=f"st{h}_{ti}")
                    for sb in range(SBL):
                        nc.tensor.matmul(
                            st[:, sb * 512:(sb + 1) * 512],
                            khT[h][0:65, ti * 128:(ti + 1) * 128],
                            qhT[h][0:65, sb * 512:(sb + 1) * 512],
                            start=True, stop=True,
                        )
                    ptile = pt_pool.tile([128, 1024], bf16, tag="pt", name=f"pt{h}_{ti}")
                    nc.scalar.activation(ptile[:], st[:], mybir.ActivationFunctionType.Exp,
                                         scale=0.125)
                    for sb in range(SBL):
                        nc.tensor.matmul(
                            oTs[sb][0:64, :],
                            vh[ti][:, h * P:(h + 1) * P],
                            ptile[:, sb * 512:(sb + 1) * 512],
                            start=(ti == 0), stop=(ti == TT - 1),
                            tile_position=(0, 0),
                        )
                        nc.tensor.matmul(
                            oTs[sb][64:65, :],
                            ones_bf[:],
                            ptile[:, sb * 512:(sb + 1) * 512],
                            start=(ti == 0), stop=(ti == TT - 1),
                            tile_position=(0, 64),
                        )
                    if h + 1 < H and ti < QT:
                        maxpass_qi(h + 1, ti)

                # --- stage oT, transpose, normalize into out_norm ---
                oT_h = oT_sb_pool.tile([65, SC], f32, tag="oT_h", name=f"oTh{h}")
                for sb in range(SBL):
                    nc.vector.tensor_copy(oT_h[:, sb * 512:(sb + 1) * 512], oTs[sb][:])
                for sc in range(QT):
                    tps = oT_ps.tile([128, 128], f32, tag="oT", name=f"tps{h}_{sc}")
                    nc.tensor.transpose(
                        tps[0:128, 0:65],
                        oT_h[:, sc * 128:(sc + 1) * 128],
                        ident[0:65, 0:65],
                    )
                    rec = small_pool.tile([128, 1], f32, tag="rec", name=f"rec{h}_{sc}")
                    nc.vector.reciprocal(rec[:], tps[:, 64:65])
                    nc.vector.tensor_scalar_mul(
                        out_norm[sc][:, h * P:(h + 1) * P], tps[:, 0:64], rec[:],
                    )
                if h % 2 == 1:
                    c = h // 2
                    for sc in range(QT):
                        tps2 = oT_ps.tile([128, 128], bf16, tag="oT", name=f"tb{c}_{sc}")
                        nc.tensor.transpose(
                            tps2[:], out_norm[sc][:, c * 128:(c + 1) * 128], ident_bf[:],
                        )
                        nc.scalar.copy(outT[c][:, sc * 128:(sc + 1) * 128], tps2[:])

        # ---- final projection ----
        with tc.tile_pool(name="fin_ps", bufs=2, space="PSUM") as fin_ps, \
             tc.tile_pool(name="fin_sb", bufs=2) as fin_sb_pool:
            for sc in range(QT):
                ps = fin_ps.tile([128, 512], f32, tag="fps", name=f"fps{sc}")
                for c in range(DCH):
                    nc.tensor.matmul(
                        ps[:],
                        outT[c][:, sc * 128:(sc + 1) * 128],
                        wo_bf[c][:],
                        start=(c == 0), stop=(c == DCH - 1),
                    )
                fin = fin_sb_pool.tile([128, 512], f32, tag="fin", name=f"fin{sc}")
                nc.vector.tensor_copy(fin[:], ps[:])
                nc.sync.dma_start(out_d[sc * 128:(sc + 1) * 128, :], fin[:])

    nc.compile()
    return nc


def kernel(q, k, v, Wq, Wk, Wv, Wo):
    nc = _build()
    from concourse.bass_utils import run_bass_kernel_spmd

    q = np.asarray(q, np.float32)
    k = np.asarray(k, np.float32)
    v = np.asarray(v, np.float32)
    in_maps = []
    for c in range(NCORES):
        b, half = c // 2, c % 2
        in_maps.append({
            "q": np.ascontiguousarray(q[b, half * SC:(half + 1) * SC, :]),
            "k": np.ascontiguousarray(k[b]),
            "v": np.ascontiguousarray(v[b]),
            "Wq": np.ascontiguousarray(Wq, dtype=np.float32),
            "Wk": np.ascontiguousarray(Wk, dtype=np.float32),
            "Wv": np.ascontiguousarray(Wv, dtype=np.float32),
            "Wo": np.ascontiguousarray(Wo, dtype=np.float32),
        })
    res = run_bass_kernel_spmd(nc, in_maps, core_ids=list(range(NCORES)))
    globals()["LAST_RES"] = res
    out = np.empty((B, S1, D), np.float32)
    for c, r in enumerate(res.results):
        b, half = c // 2, c % 2
        out[b, half * SC:(half + 1) * SC] = r["out"]
    return out


if __name__ == "__main__":
    rng = np.random.default_rng(0)
    qq = rng.standard_normal((B, S1, D), dtype=np.float32)
    kk = rng.standard_normal((B, S2, D), dtype=np.float32)
    vv = rng.standard_normal((B, S2, D), dtype=np.float32)
    wq = rng.standard_normal((H, D, P), dtype=np.float32)
    wk = rng.standard_normal((H, D, P), dtype=np.float32)
    wv = rng.standard_normal((H, D, P), dtype=np.float32)
    wo = rng.standard_normal((H * P, D), dtype=np.float32)
    o = kernel(qq, kk, vv, wq, wk, wv, wo)
    print("out", o.shape, o.dtype, np.abs(o).mean())



# revision 10
# speedup vs baseline: 1.5104x; 1.5104x over previous
"""CrossAttention TRN2 kernel v2: 8-core SPMD, shard = (batch, S1-half).

Host pre-transposes q,k,v (-> qT/kT/vT chunk arrays) and pre-packs weights, so
the device does zero layout transposes. Per core:
  1. Load qT/kT/vT (f32r, rounded at input), packed Wq/Wk/Wv (f32r), Wo (bf16,
     per-head [p, h*512+d]).
  2. Projections: qhT[h] [65,1024] / khT[h] [65,2048] (row 64 = negm / ones),
     vh_aug[ti] [128, 8*65] bf16 (per-head 64 cols + ones col -> fused PV+rowsum).
  3. Per head: raw-S max pass ([s,t] psum -> DVE negated rowmax -> gpsimd
     min/cast -> per-qi DMA into qhT row 64), S^T pass with K=65 (ones row in
     khT adds -max[s]), ACT exp(scale=1/8) -> P^T bf16, PV single matmul M=65
     accumulating oT[65,512] x2 (row 64 = softmax denominator).
     Pipelined emission: S^T(ti) | maxpass(h+1) piece | exp(ti) | PV(ti-1);
     S^T and maxpass share one 3-buf [128,1024] PSUM pool.
  4. Tail per head (spread into next head's loop): oT->SBUF (ACT), den row ->
     partition 0 via tiny DMA, DVE reciprocal, gpsimd broadcast+multiply ->
     normalized oT_hn[h] [64,1024] bf16.
  5. Final proj out[s,d] = sum_h oT_hn[h]^T @ Wo_h (K=64 per head, psum
     accumulate over heads) -> 8 store DMAs. No transposes anywhere.
"""
import sys
import functools

sys.path.insert(0, "/opt/trn_rl_repo")
import numpy as np
from contextlib import ExitStack

B, S1, S2, D, H, P = 4, 2048, 2048, 512, 8, 64
SC = S1 // 2          # 1024 q rows per core
NCORES = 8
DCH = D // 128        # 4 d-chunks
QT = SC // 128        # 8 q s-tiles
TT = S2 // 128        # 16 t-tiles


@functools.lru_cache(maxsize=1)
def _build():
    from concourse import bacc, tile, mybir

    f32 = mybir.dt.float32
    f32r = mybir.dt.float32r
    bf16 = mybir.dt.bfloat16

    nc = bacc.Bacc("TRN2", target_bir_lowering=False, debug=False)

    qT_d = nc.dram_tensor("qT", [DCH, 128, SC], f32r, kind="ExternalInput").ap()
    kT_d = nc.dram_tensor("kT", [DCH, 128, S2], f32r, kind="ExternalInput").ap()
    vT_d = nc.dram_tensor("vT", [DCH, 128, S2], f32r, kind="ExternalInput").ap()
    wq_d = nc.dram_tensor("wq", [DCH, 128, H * P], f32r, kind="ExternalInput").ap()
    wk_d = nc.dram_tensor("wk", [DCH, 128, H * P], f32r, kind="ExternalInput").ap()
    wv_d = nc.dram_tensor("wv", [DCH, 128, H * P], f32r, kind="ExternalInput").ap()
    wo_d = nc.dram_tensor("wo", [P, H * D], bf16, kind="ExternalInput").ap()
    out_d = nc.dram_tensor("out", [SC, D], f32, kind="ExternalOutput").ap()

    with tile.TileContext(nc) as tc, ExitStack() as ctx:
        # ---- persistent SBUF (allocate all tags up front) ----
        acts = ctx.enter_context(tc.tile_pool(name="acts", bufs=1))
        qhT = [acts.tile([65, SC], f32r, tag=f"qhT{h}", name=f"qhT{h}") for h in range(H)]
        khT = [acts.tile([65, S2], f32r, tag=f"khT{h}", name=f"khT{h}") for h in range(H)]
        vh = [acts.tile([128, H * 65], bf16, tag=f"vh{t}", name=f"vh{t}") for t in range(TT)]
        wo_sb = acts.tile([P, H * D], bf16, tag="wo", name="wo_sb")
        oT_hn = [acts.tile([P, SC], bf16, tag=f"ohn{h}", name=f"ohn{h}") for h in range(H)]

        small = ctx.enter_context(tc.tile_pool(name="small", bufs=4))

        # shared PSUM work pool: S^T tiles and maxpass tiles (3 x 2 banks)
        work_ps = ctx.enter_context(tc.tile_pool(name="work", bufs=3, space="PSUM"))

        # ---- ones rows of khT (gpsimd, overlapped with loads) ----
        for h in range(H):
            nc.gpsimd.memset(khT[h].bitcast(f32)[64:65, :], 1.0)

        copy_alt = [0]

        def copy_split(dst0, src0, dst1, src1):
            """Two psum->sbuf copies, alternating ACT/DVE."""
            if copy_alt[0] % 2 == 0:
                nc.scalar.copy(dst0, src0)
                nc.vector.tensor_copy(dst1, src1)
            else:
                nc.vector.tensor_copy(dst0, src0)
                nc.scalar.copy(dst1, src1)
            copy_alt[0] += 1

        # ---- maxpass pieces (head h, interleaved into other loops) ----
        nacc = [small.tile([128, 1], f32, tag=f"nacc{q % 2}", name=f"nacc{q}") for q in range(QT)]

        def maxpass_piece(h, qi, half):
            mx = work_ps.tile([128, 1024], f32, tag="work", name=f"mx{h}_{qi}_{half}")
            for tb in range(2):
                nc.tensor.matmul(
                    mx[:, tb * 512:(tb + 1) * 512],
                    qhT[h][0:64, qi * 128:(qi + 1) * 128],
                    khT[h][0:64, half * 1024 + tb * 512: half * 1024 + (tb + 1) * 512],
                    start=True, stop=True,
                )
            negm_p = small.tile([128, 1], f32, tag="negp", name=f"negp{h}_{qi}_{half}")
            nc.vector.tensor_reduce(
                negm_p[:], mx[:], axis=mybir.AxisListType.X,
                op=mybir.AluOpType.max, negate=True,
            )
            if half == 0:
                nc.gpsimd.tensor_copy(nacc[qi][:], negm_p[:])
            else:
                nc.gpsimd.tensor_scalar_min(nacc[qi][:], negm_p[:], nacc[qi][:])
                negm_r = small.tile([128, 1], f32r, tag="negr", name=f"negr{h}_{qi}")
                nc.gpsimd.tensor_copy(negm_r[:], nacc[qi][:])
                nc.sync.dma_start(
                    qhT[h][64:65, qi * 128:(qi + 1) * 128], negm_r[:],
                )

        # ---- setup: loads + projections ----
        with tc.tile_pool(name="qph", bufs=1) as qpool, \
             tc.tile_pool(name="proj_ps", bufs=2, space="PSUM") as proj_ps:
            wq_sb = qpool.tile([128, DCH * H * P], f32r, tag="wq", name="wq_sb")
            qT_sb = qpool.tile([128, DCH * SC], f32r, tag="qT", name="qT_sb")
            for c in range(DCH):
                nc.sync.dma_start(wq_sb[:, c * 512:(c + 1) * 512], wq_d[c])
            for c in range(DCH):
                nc.sync.dma_start(qT_sb[:, c * SC:(c + 1) * SC], qT_d[c])
            for hp in range(H // 2):
                for sb in range(2):
                    ps = proj_ps.tile([128, 512], f32, tag="pp", name=f"qp{hp}_{sb}")
                    for c in range(DCH):
                        nc.tensor.matmul(
                            ps[:],
                            wq_sb[:, c * 512 + hp * 128: c * 512 + (hp + 1) * 128],
                            qT_sb[:, c * SC + sb * 512: c * SC + sb * 512 + 512],
                            start=(c == 0), stop=(c == DCH - 1),
                        )
                    copy_split(
                        qhT[2 * hp][0:64, sb * 512:(sb + 1) * 512], ps[0:64, :],
                        qhT[2 * hp + 1][0:64, sb * 512:(sb + 1) * 512], ps[64:128, :],
                    )

            with tc.tile_pool(name="kph", bufs=1) as kpool:
                wk_sb = kpool.tile([128, DCH * H * P], f32r, tag="wk", name="wk_sb")
                kT_sb = kpool.tile([128, DCH * S2], f32r, tag="kT", name="kT_sb")
                for c in range(DCH):
                    nc.sync.dma_start(wk_sb[:, c * 512:(c + 1) * 512], wk_d[c])
                for c in range(DCH):
                    nc.sync.dma_start(kT_sb[:, c * S2:(c + 1) * S2], kT_d[c])
                for hp in range(H // 2):
                    for tb in range(4):
                        ps = proj_ps.tile([128, 512], f32, tag="pp", name=f"kp{hp}_{tb}")
                        for c in range(DCH):
                            nc.tensor.matmul(
                                ps[:],
                                wk_sb[:, c * 512 + hp * 128: c * 512 + (hp + 1) * 128],
                                kT_sb[:, c * S2 + tb * 512: c * S2 + tb * 512 + 512],
                                start=(c == 0), stop=(c == DCH - 1),
                            )
                        copy_split(
                            khT[2 * hp][0:64, tb * 512:(tb + 1) * 512], ps[0:64, :],
                            khT[2 * hp + 1][0:64, tb * 512:(tb + 1) * 512], ps[64:128, :],
                        )

            with tc.tile_pool(name="vph", bufs=1) as vpool:
                wv_sb = vpool.tile([128, DCH * H * P], f32r, tag="wv", name="wv_sb")
                vT_sb = vpool.tile([128, DCH * S2], f32r, tag="vT", name="vT_sb")
                for c in range(DCH):
                    nc.sync.dma_start(wv_sb[:, c * 512:(c + 1) * 512], wv_d[c])
                for c in range(DCH):
                    nc.sync.dma_start(vT_sb[:, c * S2:(c + 1) * S2], vT_d[c])
                nc.sync.dma_start(wo_sb[:], wo_d)
                # v-proj with maxpass(0) interleaved (one piece per t-tile)
                for ti in range(TT):
                    ps = proj_ps.tile([128, 512], f32, tag="pp", name=f"vp{ti}")
                    for c in range(DCH):
                        nc.tensor.matmul(
                            ps[:],
                            vT_sb[:, c * S2 + ti * 128: c * S2 + (ti + 1) * 128],
                            wv_sb[:, c * 512:(c + 1) * 512],
                            start=(c == 0), stop=(c == DCH - 1),
                        )
                    vdst = vh[ti][:].rearrange("t (h q) -> t h q", h=H, q=65)
                    eng = nc.scalar if ti % 2 == 0 else nc.vector
                    if ti % 2 == 0:
                        nc.scalar.copy(vdst[:, :, 0:64],
                                       ps[:].rearrange("t (h q) -> t h q", h=H, q=64))
                    else:
                        nc.vector.tensor_copy(vdst[:, :, 0:64],
                                              ps[:].rearrange("t (h q) -> t h q", h=H, q=64))
                    nc.gpsimd.memset(vdst[:, :, 64:65], 1.0)
                    maxpass_piece(0, ti // 2, ti % 2)

        # ---- attention-phase pools (opened after setup pools freed) ----
        tail_pool = ctx.enter_context(tc.tile_pool(name="tail", bufs=1))
        oT_h_t = [tail_pool.tile([65, SC], f32, tag=f"oth{i}", name=f"oth{i}") for i in range(2)]
        dn_t = [tail_pool.tile([1, SC], f32, tag=f"dn{i}", name=f"dn{i}") for i in range(2)]
        rec_t = [tail_pool.tile([1, SC], f32, tag=f"rec{i}", name=f"rec{i}") for i in range(2)]
        recb_t = [tail_pool.tile([P, SC], f32, tag=f"recb{i}", name=f"recb{i}") for i in range(2)]
        pt_pool = ctx.enter_context(tc.tile_pool(name="pt", bufs=3))
        fin_pool = ctx.enter_context(tc.tile_pool(name="fin", bufs=2))

        # ---- attention ----
        oT_tiles = {}

        def pv(h, tj, ptile):
            for sb in range(2):
                nc.tensor.matmul(
                    oT_tiles[h][sb][0:65, :],
                    vh[tj][:, h * 65:(h + 1) * 65],
                    ptile[:, sb * 512:(sb + 1) * 512],
                    start=(tj == 0), stop=(tj == TT - 1),
                )

        def tail_copy(h):
            """oT psum -> SBUF staging (emit right after PV(15) of head h)."""
            i = h % 2
            oT_h, oTs = oT_h_t[i], oT_tiles[h]
            nc.scalar.copy(oT_h[:, 0:512], oTs[0][:])
            nc.scalar.copy(oT_h[:, 512:1024], oTs[1][:])

        def emit_tail(h):
            """Normalize head h's oT_h (spread into next head's loop)."""
            i = h % 2
            oT_h, dn, rec, recb = oT_h_t[i], dn_t[i], rec_t[i], recb_t[i]

            def p1():
                nc.sync.dma_start(dn[0:1, :], oT_h[64:65, :])
            def p2():
                nc.vector.reciprocal(rec[0:1, :], dn[0:1, :])
            def p3():
                nc.gpsimd.partition_broadcast(recb[0:P, :], rec[0:1, :], channels=P)
            def p4():
                nc.gpsimd.tensor_tensor(
                    oT_hn[h][0:P, :], oT_h[0:P, :], recb[0:P, :],
                    op=mybir.AluOpType.mult,
                )
            return [p1, p2, p3, p4]

        with tc.tile_pool(name="oT_ps", bufs=1, space="PSUM") as oT_ps:
            for h in range(H):
                oT_tiles[h] = [
                    oT_ps.tile([65, 512], f32, tag=f"oT{sb}", name=f"oT{h}_{sb}")
                    for sb in range(2)
                ]
                pts = {}
                tail_cl = emit_tail(h - 1) if h > 0 else []
                for ti in range(TT):
                    st = work_ps.tile([128, 1024], f32, tag="work", name=f"st{h}_{ti}")
                    for sb in range(2):
                        nc.tensor.matmul(
                            st[:, sb * 512:(sb + 1) * 512],
                            khT[h][0:65, ti * 128:(ti + 1) * 128],
                            qhT[h][0:65, sb * 512:(sb + 1) * 512],
                            start=True, stop=True,
                        )
                    if h + 1 < H:
                        maxpass_piece(h + 1, ti // 2, ti % 2)
                    ptile = pt_pool.tile([128, 1024], bf16, tag="pt", name=f"pt{h}_{ti}")
                    nc.scalar.activation(ptile[:], st[:], mybir.ActivationFunctionType.Exp,
                                         scale=0.125)
                    pts[ti] = ptile
                    if ti > 0:
                        pv(h, ti - 1, pts[ti - 1])
                    if 0 <= ti - 1 < len(tail_cl):
                        tail_cl[ti - 1]()
                pv(h, TT - 1, pts[TT - 1])
                tail_copy(h)
            # head 7 tail normalize runs here
            for cl in emit_tail(H - 1):
                cl()

        # ---- final projection: out[s,d] = sum_h oT_hn[h]^T @ Wo_h ----
        with tc.tile_pool(name="fin_ps", bufs=2, space="PSUM") as fin_ps:
            for sc in range(QT):
                fp = fin_ps.tile([128, 512], f32, tag="fp", name=f"fp{sc}")
                for h in range(H):
                    nc.tensor.matmul(
                        fp[:],
                        oT_hn[h][0:P, sc * 128:(sc + 1) * 128],
                        wo_sb[0:P, h * D:(h + 1) * D],
                        start=(h == 0), stop=(h == H - 1),
                    )
                fin = fin_pool.tile([128, 512], f32, tag="fin", name=f"fin{sc}")
                if sc % 2 == 0:
                    nc.vector.tensor_copy(fin[:], fp[:])
                else:
                    nc.scalar.copy(fin[:], fp[:])
                nc.sync.dma_start(out_d[sc * 128:(sc + 1) * 128, :], fin[:])

    nc.compile()
    return nc


def _host_prep(q, k, v, Wq, Wk, Wv, Wo):
    import ml_dtypes
    wq_a = np.ascontiguousarray(
        Wq.transpose(1, 0, 2).reshape(DCH, 128, H * P), dtype=np.float32)
    wk_a = np.ascontiguousarray(
        Wk.transpose(1, 0, 2).reshape(DCH, 128, H * P), dtype=np.float32)
    wv_a = np.ascontiguousarray(
        Wv.transpose(1, 0, 2).reshape(DCH, 128, H * P), dtype=np.float32)
    wo_a = np.ascontiguousarray(
        Wo.reshape(H, P, D).transpose(1, 0, 2).reshape(P, H * D)
    ).astype(ml_dtypes.bfloat16)
    in_maps = []
    for c in range(NCORES):
        b, half = c // 2, c % 2
        qT = np.ascontiguousarray(
            q[b, half * SC:(half + 1) * SC, :].T.reshape(DCH, 128, SC))
        kT = np.ascontiguousarray(k[b].T.reshape(DCH, 128, S2))
        vT = np.ascontiguousarray(v[b].T.reshape(DCH, 128, S2))
        in_maps.append({
            "qT": qT, "kT": kT, "vT": vT,
            "wq": wq_a, "wk": wk_a, "wv": wv_a, "wo": wo_a,
        })
    return in_maps


def kernel(q, k, v, Wq, Wk, Wv, Wo):
    nc = _build()
    from concourse.bass_utils import run_bass_kernel_spmd

    q = np.asarray(q, np.float32)
    k = np.asarray(k, np.float32)
    v = np.asarray(v, np.float32)
    in_maps = _host_prep(q, k, v, np.asarray(Wq, np.float32),
                         np.asarray(Wk, np.float32), np.asarray(Wv, np.float32),
                         np.asarray(Wo, np.float32))
    res = run_bass_kernel_spmd(nc, in_maps, core_ids=list(range(NCORES)))
    globals()["LAST_RES"] = res
    out = np.empty((B, S1, D), np.float32)
    for c, r in enumerate(res.results):
        b, half = c // 2, c % 2
        out[b, half * SC:(half + 1) * SC] = r["out"]
    return out


if __name__ == "__main__":
    rng = np.random.default_rng(0)
    qq = rng.standard_normal((B, S1, D), dtype=np.float32)
    kk = rng.standard_normal((B, S2, D), dtype=np.float32)
    vv = rng.standard_normal((B, S2, D), dtype=np.float32)
    wq = rng.standard_normal((H, D, P), dtype=np.float32)
    wk = rng.standard_normal((H, D, P), dtype=np.float32)
    wv = rng.standard_normal((H, D, P), dtype=np.float32)
    wo = rng.standard_normal((H * P, D), dtype=np.float32)
    o = kernel(qq, kk, vv, wq, wk, wv, wo)
    print("out", o.shape, o.dtype, np.abs(o).mean())


# revision 12
# speedup vs baseline: 1.5586x; 1.0319x over previous
"""CrossAttention TRN2 kernel v2: 8-core SPMD, shard = (batch, S1-half).

Host pre-transposes q,k,v (-> qT/kT/vT chunk arrays) and pre-packs weights, so
the device does zero layout transposes. Per core:
  1. Load qT/kT/vT (f32r, rounded at input), packed Wq/Wk/Wv (f32r), Wo (bf16,
     per-head [p, h*512+d]).
  2. Projections: qhT[h] [65,1024] / khT[h] [65,2048] (row 64 = negm / ones),
     vh_aug[ti] [128, 8*65] bf16 (per-head 64 cols + ones col -> fused PV+rowsum).
  3. Per head: raw-S max pass ([s,t] psum -> DVE negated rowmax -> gpsimd
     min/cast -> per-qi DMA into qhT row 64), S^T pass with K=65 (ones row in
     khT adds -max[s]), ACT exp(scale=1/8) -> P^T bf16, PV single matmul M=65
     accumulating oT[65,512] x2 (row 64 = softmax denominator).
     Pipelined emission: S^T(ti) | maxpass(h+1) piece | exp(ti) | PV(ti-1);
     S^T and maxpass share one 3-buf [128,1024] PSUM pool.
  4. Tail per head (spread into next head's loop): oT->SBUF (ACT), den row ->
     partition 0 via tiny DMA, DVE reciprocal, gpsimd broadcast+multiply ->
     normalized oT_hn[h] [64,1024] bf16.
  5. Final proj out[s,d] = sum_h oT_hn[h]^T @ Wo_h (K=64 per head, psum
     accumulate over heads) -> 8 store DMAs. No transposes anywhere.
"""
import sys
import functools

sys.path.insert(0, "/opt/trn_rl_repo")
import numpy as np
from contextlib import ExitStack

B, S1, S2, D, H, P = 4, 2048, 2048, 512, 8, 64
SC = S1 // 2          # 1024 q rows per core
NCORES = 8
DCH = D // 128        # 4 d-chunks
QT = SC // 128        # 8 q s-tiles
TT = S2 // 128        # 16 t-tiles


@functools.lru_cache(maxsize=1)
def _build():
    from concourse import bacc, tile, mybir

    f32 = mybir.dt.float32
    f32r = mybir.dt.float32r
    bf16 = mybir.dt.bfloat16

    nc = bacc.Bacc("TRN2", target_bir_lowering=False, debug=False)

    qT_d = nc.dram_tensor("qT", [DCH, 128, SC], f32r, kind="ExternalInput").ap()
    kT_d = nc.dram_tensor("kT", [DCH, 128, S2], f32r, kind="ExternalInput").ap()
    vT_d = nc.dram_tensor("vT", [DCH, 128, S2], f32r, kind="ExternalInput").ap()
    wq_d = nc.dram_tensor("wq", [DCH, 128, H * P], f32r, kind="ExternalInput").ap()
    wk_d = nc.dram_tensor("wk", [DCH, 128, H * P], f32r, kind="ExternalInput").ap()
    wv_d = nc.dram_tensor("wv", [DCH, 128, H * P], f32r, kind="ExternalInput").ap()
    wo_d = nc.dram_tensor("wo", [P, H * D], bf16, kind="ExternalInput").ap()
    out_d = nc.dram_tensor("out", [SC, D], f32, kind="ExternalOutput").ap()

    with tile.TileContext(nc) as tc, ExitStack() as ctx:
        # ---- persistent SBUF (allocate all tags up front) ----
        acts = ctx.enter_context(tc.tile_pool(name="acts", bufs=1))
        qhT = [acts.tile([65, SC], f32r, tag=f"qhT{h}", name=f"qhT{h}") for h in range(H)]
        khT = [acts.tile([65, S2], f32r, tag=f"khT{h}", name=f"khT{h}") for h in range(H)]
        vh = [acts.tile([128, H * 65], bf16, tag=f"vh{t}", name=f"vh{t}") for t in range(TT)]
        wo_sb = acts.tile([P, H * D], bf16, tag="wo", name="wo_sb")
        oT_hn = [acts.tile([P, SC], bf16, tag=f"ohn{h}", name=f"ohn{h}") for h in range(H)]

        small = ctx.enter_context(tc.tile_pool(name="small", bufs=4))

        # shared PSUM work pool: S^T tiles and maxpass tiles (3 x 2 banks)
        work_ps = ctx.enter_context(tc.tile_pool(name="work", bufs=3, space="PSUM"))

        # ---- ones rows of khT (gpsimd, overlapped with loads) ----
        for h in range(H):
            nc.gpsimd.memset(khT[h].bitcast(f32)[64:65, :], 1.0)

        copy_alt = [0]

        def copy_split(dst0, src0, dst1, src1):
            """Two psum->sbuf copies, alternating ACT/DVE."""
            if copy_alt[0] % 2 == 0:
                nc.scalar.copy(dst0, src0)
                nc.vector.tensor_copy(dst1, src1)
            else:
                nc.vector.tensor_copy(dst0, src0)
                nc.scalar.copy(dst1, src1)
            copy_alt[0] += 1

        # ---- maxpass pieces (head h, interleaved into other loops) ----
        nacc = [small.tile([128, 1], f32, tag=f"nacc{q % 2}", name=f"nacc{q}") for q in range(QT)]

        def maxpass_piece(h, qi, half):
            mx = work_ps.tile([128, 1024], f32, tag="work", name=f"mx{h}_{qi}_{half}")
            for tb in range(2):
                nc.tensor.matmul(
                    mx[:, tb * 512:(tb + 1) * 512],
                    qhT[h][0:64, qi * 128:(qi + 1) * 128],
                    khT[h][0:64, half * 1024 + tb * 512: half * 1024 + (tb + 1) * 512],
                    start=True, stop=True,
                )
            negm_p = small.tile([128, 1], f32, tag="negp", name=f"negp{h}_{qi}_{half}")
            nc.vector.tensor_reduce(
                negm_p[:], mx[:], axis=mybir.AxisListType.X,
                op=mybir.AluOpType.max, negate=True,
            )
            if half == 0:
                nc.gpsimd.tensor_copy(nacc[qi][:], negm_p[:])
            else:
                nc.gpsimd.tensor_scalar_min(nacc[qi][:], negm_p[:], nacc[qi][:])
                negm_r = small.tile([128, 1], f32r, tag="negr", name=f"negr{h}_{qi}")
                nc.gpsimd.tensor_copy(negm_r[:], nacc[qi][:])
                nc.sync.dma_start(
                    qhT[h][64:65, qi * 128:(qi + 1) * 128], negm_r[:],
                )

        # ---- setup: loads + projections ----
        with tc.tile_pool(name="qph", bufs=1) as qpool, \
             tc.tile_pool(name="proj_ps", bufs=2, space="PSUM") as proj_ps:
            wq_sb = qpool.tile([128, DCH * H * P], f32r, tag="wq", name="wq_sb")
            qT_sb = qpool.tile([128, DCH * SC], f32r, tag="qT", name="qT_sb")
            for c in range(DCH):
                nc.sync.dma_start(wq_sb[:, c * 512:(c + 1) * 512], wq_d[c])
            for c in range(DCH):
                nc.sync.dma_start(qT_sb[:, c * SC:(c + 1) * SC], qT_d[c])
            for hp in range(H // 2):
                for sb in range(2):
                    ps = proj_ps.tile([128, 512], f32, tag="pp", name=f"qp{hp}_{sb}")
                    for c in range(DCH):
                        nc.tensor.matmul(
                            ps[:],
                            wq_sb[:, c * 512 + hp * 128: c * 512 + (hp + 1) * 128],
                            qT_sb[:, c * SC + sb * 512: c * SC + sb * 512 + 512],
                            start=(c == 0), stop=(c == DCH - 1),
                        )
                    copy_split(
                        qhT[2 * hp][0:64, sb * 512:(sb + 1) * 512], ps[0:64, :],
                        qhT[2 * hp + 1][0:64, sb * 512:(sb + 1) * 512], ps[64:128, :],
                    )

            with tc.tile_pool(name="kph", bufs=1) as kpool:
                wk_sb = kpool.tile([128, DCH * H * P], f32r, tag="wk", name="wk_sb")
                kT_sb = kpool.tile([128, DCH * S2], f32r, tag="kT", name="kT_sb")
                for c in range(DCH):
                    nc.sync.dma_start(wk_sb[:, c * 512:(c + 1) * 512], wk_d[c])
                for c in range(DCH):
                    nc.sync.dma_start(kT_sb[:, c * S2:(c + 1) * S2], kT_d[c])
                for hp in range(H // 2):
                    for tb in range(4):
                        ps = proj_ps.tile([128, 512], f32, tag="pp", name=f"kp{hp}_{tb}")
                        for c in range(DCH):
                            nc.tensor.matmul(
                                ps[:],
                                wk_sb[:, c * 512 + hp * 128: c * 512 + (hp + 1) * 128],
                                kT_sb[:, c * S2 + tb * 512: c * S2 + tb * 512 + 512],
                                start=(c == 0), stop=(c == DCH - 1),
                            )
                        copy_split(
                            khT[2 * hp][0:64, tb * 512:(tb + 1) * 512], ps[0:64, :],
                            khT[2 * hp + 1][0:64, tb * 512:(tb + 1) * 512], ps[64:128, :],
                        )

            with tc.tile_pool(name="vph", bufs=1) as vpool:
                wv_sb = vpool.tile([128, DCH * H * P], f32r, tag="wv", name="wv_sb")
                vT_sb = vpool.tile([128, DCH * S2], f32r, tag="vT", name="vT_sb")
                for c in range(DCH):
                    nc.sync.dma_start(wv_sb[:, c * 512:(c + 1) * 512], wv_d[c])
                for c in range(DCH):
                    nc.sync.dma_start(vT_sb[:, c * S2:(c + 1) * S2], vT_d[c])
                nc.sync.dma_start(wo_sb[:], wo_d)
                # v-proj with maxpass(0) interleaved (one piece per t-tile)
                for ti in range(TT):
                    ps = proj_ps.tile([128, 512], f32, tag="pp", name=f"vp{ti}")
                    for c in range(DCH):
                        nc.tensor.matmul(
                            ps[:],
                            vT_sb[:, c * S2 + ti * 128: c * S2 + (ti + 1) * 128],
                            wv_sb[:, c * 512:(c + 1) * 512],
                            start=(c == 0), stop=(c == DCH - 1),
                        )
                    vdst = vh[ti][:].rearrange("t (h q) -> t h q", h=H, q=65)
                    eng = nc.scalar if ti % 2 == 0 else nc.vector
                    if ti % 2 == 0:
                        nc.scalar.copy(vdst[:, :, 0:64],
                                       ps[:].rearrange("t (h q) -> t h q", h=H, q=64))
                    else:
                        nc.vector.tensor_copy(vdst[:, :, 0:64],
                                              ps[:].rearrange("t (h q) -> t h q", h=H, q=64))
                    nc.gpsimd.memset(vdst[:, :, 64:65], 1.0)
                    maxpass_piece(0, ti // 2, ti % 2)
                # head 1's qi=0 pieces belong to "ti 14/15 of head -1" = here
                maxpass_piece(1, 0, 0)
                maxpass_piece(1, 0, 1)

        # ---- attention-phase pools (opened after setup pools freed) ----
        tail_pool = ctx.enter_context(tc.tile_pool(name="tail", bufs=1))
        oT_h_t = [tail_pool.tile([65, SC], f32, tag=f"oth{i}", name=f"oth{i}") for i in range(2)]
        dn_t = [tail_pool.tile([1, SC], f32, tag=f"dn{i}", name=f"dn{i}") for i in range(2)]
        rec_t = [tail_pool.tile([1, SC], f32, tag=f"rec{i}", name=f"rec{i}") for i in range(2)]
        recb_t = [tail_pool.tile([P, SC], f32, tag=f"recb{i}", name=f"recb{i}") for i in range(2)]
        pt_pool = ctx.enter_context(tc.tile_pool(name="pt", bufs=3))
        fin_pool = ctx.enter_context(tc.tile_pool(name="fin", bufs=2))

        # ---- attention ----
        oT_tiles = {}

        def pv(h, tj, ptile):
            for sb in range(2):
                nc.tensor.matmul(
                    oT_tiles[h][sb][0:65, :],
                    vh[tj][:, h * 65:(h + 1) * 65],
                    ptile[:, sb * 512:(sb + 1) * 512],
                    start=(tj == 0), stop=(tj == TT - 1),
                )

        def tail_copy(h):
            """oT psum -> SBUF staging (emit right after PV(15) of head h)."""
            i = h % 2
            oT_h, oTs = oT_h_t[i], oT_tiles[h]
            nc.scalar.copy(oT_h[:, 0:512], oTs[0][:])
            nc.scalar.copy(oT_h[:, 512:1024], oTs[1][:])

        def emit_tail(h):
            """Normalize head h's oT_h (spread into next head's loop)."""
            i = h % 2
            oT_h, dn, rec, recb = oT_h_t[i], dn_t[i], rec_t[i], recb_t[i]

            def p1():
                nc.sync.dma_start(dn[0:1, :], oT_h[64:65, :])
            def p2():
                nc.vector.reciprocal(rec[0:1, :], dn[0:1, :])
            def p3():
                nc.gpsimd.partition_broadcast(recb[0:P, :], rec[0:1, :], channels=P)
            def p4():
                nc.gpsimd.tensor_tensor(
                    oT_hn[h][0:P, :], oT_h[0:P, :], recb[0:P, :],
                    op=mybir.AluOpType.mult,
                )
            return [p1, p2, p3, p4]

        with tc.tile_pool(name="oT_ps", bufs=1, space="PSUM") as oT_ps:
            for h in range(H):
                oT_tiles[h] = [
                    oT_ps.tile([65, 512], f32, tag=f"oT{sb}", name=f"oT{h}_{sb}")
                    for sb in range(2)
                ]
                pts = {}
                tail_cl = emit_tail(h - 1) if h > 0 else []
                for ti in range(TT):
                    st = work_ps.tile([128, 1024], f32, tag="work", name=f"st{h}_{ti}")
                    for sb in range(2):
                        nc.tensor.matmul(
                            st[:, sb * 512:(sb + 1) * 512],
                            khT[h][0:65, ti * 128:(ti + 1) * 128],
                            qhT[h][0:65, sb * 512:(sb + 1) * 512],
                            start=True, stop=True,
                        )
                    # maxpass pieces shifted 2 slots early: head h+1's qi=0
                    # was emitted at ti 14/15 of head h-1, so the last negm
                    # DMA (qi=7) issues at ti=13 and its latency hides.
                    if h + 1 < H and ti <= 13:
                        maxpass_piece(h + 1, (ti + 2) // 2, (ti + 2) % 2)
                    if h + 2 < H and ti >= 14:
                        maxpass_piece(h + 2, 0, ti - 14)
                    ptile = pt_pool.tile([128, 1024], bf16, tag="pt", name=f"pt{h}_{ti}")
                    nc.scalar.activation(ptile[:], st[:], mybir.ActivationFunctionType.Exp,
                                         scale=0.125)
                    pts[ti] = ptile
                    if ti > 0:
                        pv(h, ti - 1, pts[ti - 1])
                    if 0 <= ti - 1 < len(tail_cl):
                        tail_cl[ti - 1]()
                pv(h, TT - 1, pts[TT - 1])
                tail_copy(h)
            # head 7 tail normalize runs here
            for cl in emit_tail(H - 1):
                cl()

        # ---- final projection: out[s,d] = sum_h oT_hn[h]^T @ Wo_h ----
        with tc.tile_pool(name="fin_ps", bufs=2, space="PSUM") as fin_ps:
            for sc in range(QT):
                fp = fin_ps.tile([128, 512], f32, tag="fp", name=f"fp{sc}")
                for h in range(H):
                    nc.tensor.matmul(
                        fp[:],
                        oT_hn[h][0:P, sc * 128:(sc + 1) * 128],
                        wo_sb[0:P, h * D:(h + 1) * D],
                        start=(h == 0), stop=(h == H - 1),
                    )
                fin = fin_pool.tile([128, 512], f32, tag="fin", name=f"fin{sc}")
                if sc % 2 == 0:
                    nc.vector.tensor_copy(fin[:], fp[:])
                else:
                    nc.scalar.copy(fin[:], fp[:])
                nc.sync.dma_start(out_d[sc * 128:(sc + 1) * 128, :], fin[:])

    nc.compile()
    return nc


def _host_prep(q, k, v, Wq, Wk, Wv, Wo):
    import ml_dtypes
    wq_a = np.ascontiguousarray(
        Wq.transpose(1, 0, 2).reshape(DCH, 128, H * P), dtype=np.float32)
    wk_a = np.ascontiguousarray(
        Wk.transpose(1, 0, 2).reshape(DCH, 128, H * P), dtype=np.float32)
    wv_a = np.ascontiguousarray(
        Wv.transpose(1, 0, 2).reshape(DCH, 128, H * P), dtype=np.float32)
    wo_a = np.ascontiguousarray(
        Wo.reshape(H, P, D).transpose(1, 0, 2).reshape(P, H * D)
    ).astype(ml_dtypes.bfloat16)
    in_maps = []
    for c in range(NCORES):
        b, half = c // 2, c % 2
        qT = np.ascontiguousarray(
            q[b, half * SC:(half + 1) * SC, :].T.reshape(DCH, 128, SC))
        kT = np.ascontiguousarray(k[b].T.reshape(DCH, 128, S2))
        vT = np.ascontiguousarray(v[b].T.reshape(DCH, 128, S2))
        in_maps.append({
            "qT": qT, "kT": kT, "vT": vT,
            "wq": wq_a, "wk": wk_a, "wv": wv_a, "wo": wo_a,
        })
    return in_maps


def kernel(q, k, v, Wq, Wk, Wv, Wo):
    nc = _build()
    from concourse.bass_utils import run_bass_kernel_spmd

    q = np.asarray(q, np.float32)
    k = np.asarray(k, np.float32)
    v = np.asarray(v, np.float32)
    in_maps = _host_prep(q, k, v, np.asarray(Wq, np.float32),
                         np.asarray(Wk, np.float32), np.asarray(Wv, np.float32),
                         np.asarray(Wo, np.float32))
    res = run_bass_kernel_spmd(nc, in_maps, core_ids=list(range(NCORES)))
    globals()["LAST_RES"] = res
    out = np.empty((B, S1, D), np.float32)
    for c, r in enumerate(res.results):
        b, half = c // 2, c % 2
        out[b, half * SC:(half + 1) * SC] = r["out"]
    return out


if __name__ == "__main__":
    rng = np.random.default_rng(0)
    qq = rng.standard_normal((B, S1, D), dtype=np.float32)
    kk = rng.standard_normal((B, S2, D), dtype=np.float32)
    vv = rng.standard_normal((B, S2, D), dtype=np.float32)
    wq = rng.standard_normal((H, D, P), dtype=np.float32)
    wk = rng.standard_normal((H, D, P), dtype=np.float32)
    wv = rng.standard_normal((H, D, P), dtype=np.float32)
    wo = rng.standard_normal((H * P, D), dtype=np.float32)
    o = kernel(qq, kk, vv, wq, wk, wv, wo)
    print("out", o.shape, o.dtype, np.abs(o).mean())
